# revision 24
# baseline (speedup 1.0000x reference)
"""Trainium2 Bass kernel for a soft-MoE (MANN) block.

Reference math (per token b):
    g  = elu(x_gate @ g1_w.T + g1_b); g = elu(g @ g2_w.T + g2_b)
    ew = softmax(g @ g3_w.T + g3_b)                      # [B, K=8]
    h1 = elu(sum_k ew_k * (x_main @ W1_k.T) + ew @ b1)   # [B, 1024]
    h2 = elu(sum_k ew_k * (h1 @ W2_k.T) + ew @ b2)       # [B, 1024]
    y  =     sum_k ew_k * (h2 @ W3_k.T) + ew @ b3        # [B, 640]

Strategy: data-parallel over 8 NeuronCores (128 batch rows per core).
The expert combine is decomposed into a mean + deviation form:

    sum_k ew_k (x @ W_k) = x @ Wbar + sum_k c_k (x @ W_k),
    Wbar = mean_k W_k,  c_k = ew_k - 1/8.

The mean term (carrying ~94% of the signal) streams in bf16; the
deviation weights stream in fp8 e3m4 (1 byte), scaled by a per-layer
power-of-two s_w so they sit in e3m4's normal range. Since |c_k| <~
0.15, the fp8 quantization error only touches the small deviation
term, keeping overall rel-err ~0.7% while cutting weight DMA traffic
from 35 MB (bf16) to 22 MB per core. All matmuls accumulate in one
fp32 PSUM group per output chunk; the 1/s_w scale-back is folded into
the ELU evaluation. Gating runs entirely in fp32, its parameters
packed into a single DMA blob so it starts immediately.

Schedule: deviation weights are DMA'd per (expert, output-chunk) so
each 512-wide PSUM group closes as soon as its own 64 matmuls are
done, letting the ELU/transpose/rescale for chunk 0 overlap chunk 1's
matmuls and the next layer's weight streaming.
"""

import sys

sys.path.insert(0, "/opt/trn_rl_repo")

from contextlib import ExitStack

import numpy as np
import ml_dtypes

import concourse.bass as bass
from concourse import bacc
import concourse.tile as tile
from concourse import mybir
from concourse.bass_utils import run_bass_kernel_spmd
from concourse.masks import make_identity

F32 = mybir.dt.float32
BF16 = mybir.dt.bfloat16
E3 = mybir.dt.float8e3
AF = mybir.ActivationFunctionType
OP = mybir.AluOpType

B = 1024
X_MAIN, X_GATE, Y_DIM = 480, 128, 640
HID, GHID, K = 1024, 64, 8
NCORES = 8
BS = B // NCORES  # 128 batch rows per core

# trunk layer configs: (partition size of i-tiles, #i-tiles, O, o-chunk sizes)
L1 = (120, 4, HID, (512, 512))
L2 = (128, 8, HID, (512, 512))
L3 = (128, 8, Y_DIM, (512, 128))

# gating blob column layout (partition dim 128, f32):
#   cols 0:BS            xg           [X_GATE=128, BS]
#   cols BS:BS+64        g1w          [128, 64]
#   cols BS+64:BS+128    g2w on rows 0:64
#   cols BS+128:BS+136   g3w on rows 0:64
#   col  BS+136          g1b on rows 0:64
#   col  BS+137          g2b' on rows 0:64   (g2b - g2w.sum(1))
#   row 0, cols BS+138:BS+146   g3b' (g3b - g3w.sum(1))
GBLOB_COLS = BS + 146


def _build_program(with_bias: tuple[bool, bool, bool], inv_sw: tuple) -> bass.Bass:
    nc = bacc.Bacc()

    gb_ext = nc.declare_dram_parameter("gb", [128, GBLOB_COLS], F32, isOutput=False)
    xm_ext = nc.declare_dram_parameter("xm", [120, 4, BS], F32, isOutput=False)
    w_ext = []  # fp8 deviation weights, scaled by s_w
    s_ext = []  # bf16 mean weights, scaled by s_w
    b_ext = []
    for li, (P, IT, O, _) in enumerate((L1, L2, L3)):
        w_ext.append(
            nc.declare_dram_parameter(f"w{li + 1}", [K, P, IT, O], E3, isOutput=False)
        )
        s_ext.append(
            nc.declare_dram_parameter(f"s{li + 1}", [P, IT, O], BF16, isOutput=False)
        )
        if with_bias[li]:
            b_ext.append(
                nc.declare_dram_parameter(f"b{li + 1}", [K, O], F32, isOutput=False)
            )
        else:
            b_ext.append(None)
    y_ext = nc.declare_dram_parameter("y", [BS, Y_DIM], F32, isOutput=True)

    with TileCtx(nc) as tc, ExitStack() as ctx:
        const = ctx.enter_context(tc.tile_pool(name="const", bufs=1))
        gat = ctx.enter_context(tc.tile_pool(name="gat", bufs=1))
        spsum = ctx.enter_context(tc.tile_pool(name="spsum", bufs=2, space="PSUM"))
        zpsum = ctx.enter_context(tc.tile_pool(name="zpsum", bufs=4, space="PSUM"))
        tpsum = ctx.enter_context(tc.tile_pool(name="tpsum", bufs=2, space="PSUM"))
        xpool = ctx.enter_context(tc.tile_pool(name="xpool", bufs=1))
        xbp = ctx.enter_context(tc.tile_pool(name="xbp", bufs=1))
        xkp = ctx.enter_context(tc.tile_pool(name="xkp", bufs=2))
        hscr = ctx.enter_context(tc.tile_pool(name="hscr", bufs=1))
        hpool = ctx.enter_context(tc.tile_pool(name="hpool", bufs=2))
        sp = [
            ctx.enter_context(tc.tile_pool(name="s1p", bufs=1)),
            ctx.enter_context(tc.tile_pool(name="s2p", bufs=1)),
            ctx.enter_context(tc.tile_pool(name="s3p", bufs=1)),
        ]
        wp = [
            ctx.enter_context(tc.tile_pool(name="w1p", bufs=8)),
            ctx.enter_context(tc.tile_pool(name="w2p", bufs=8)),
            ctx.enter_context(tc.tile_pool(name="w3p", bufs=8)),
        ]

        # ---- gating blob first: the whole gating chain depends on it ----
        gb_sb = gat.tile([128, GBLOB_COLS], F32)
        nc.sync.dma_start(gb_sb, gb_ext[:])
        xg_sb = gb_sb[:, 0:BS]
        g1w_sb = gb_sb[:, BS : BS + 64]
        g2w_sb = gb_sb[0:64, BS + 64 : BS + 128]
        g3w_sb = gb_sb[0:64, BS + 128 : BS + 136]
        g1b_sb = gb_sb[0:64, BS + 136 : BS + 137]
        g2b_sb = gb_sb[0:64, BS + 137 : BS + 138]
        g3b_sb = gb_sb[0:1, BS + 138 : BS + 146]

        ident = const.tile([128, 128], F32)
        make_identity(nc, ident)
        identb = const.tile([128, 128], BF16)
        nc.vector.tensor_copy(out=identb, in_=ident)
        ones = const.tile([1, BS], F32)
        nc.vector.memset(ones, 1.0)

        # ---- main input + layer-1 mean weights (split per chunk) ----
        x1_sb = xpool.tile([120, 4, BS], F32, tag="x1")
        nc.sync.dma_start(x1_sb, xm_ext[:])
        s_sb = []
        for li, (P, IT, O, chunks) in enumerate((L1, L2, L3)):
            s_sb.append(
                sp[li].tile([P, IT, O], BF16, tag=f"s{li}", name=f"s{li}_sb")
            )
        # s1 halves immediately; s2/s3 are emitted later in the layer loop.
        nc.sync.dma_start(s_sb[0][:, :, 0:512], s_ext[0][:, :, 0:512])
        nc.sync.dma_start(s_sb[0][:, :, 512:1024], s_ext[0][:, :, 512:1024])

        # layer-1 input in bf16
        xb1 = xbp.tile([120, 4, BS], BF16, tag="xb1")
        nc.vector.tensor_copy(out=xb1, in_=x1_sb)

        # ---------------- gating (fp32) ----------------
        def g_ap(t):
            return t[:, 0:1]

        def gate_elup(zp, bias_sb, name):
            # returns elu(z + bias) + 1 = relu(z+bias) + exp(min(z+bias, 0))
            r = gat.tile([GHID, BS], F32, tag=f"r_{name}")
            nc.scalar.activation(r, zp, AF.Relu, bias=g_ap(bias_sb))
            m = gat.tile([GHID, BS], F32, tag=f"m_{name}")
            nc.vector.tensor_scalar(m, zp, g_ap(bias_sb), 0.0, OP.add, OP.min)
            e = gat.tile([GHID, BS], F32, tag=f"e_{name}")
            nc.scalar.activation(e, m, AF.Exp)
            hp = gat.tile([GHID, BS], F32, tag=f"hp_{name}")
            nc.vector.tensor_tensor(hp, r, e, OP.add)
            return hp

        zg1 = spsum.tile([GHID, BS], F32, tag="g")
        nc.tensor.matmul(zg1, lhsT=g1w_sb, rhs=xg_sb, start=True, stop=True)
        h1p = gate_elup(zg1, g1b_sb, "g1")

        zg2 = spsum.tile([GHID, BS], F32, tag="g")
        nc.tensor.matmul(zg2, lhsT=g2w_sb, rhs=h1p, start=True, stop=True)
        h2p = gate_elup(zg2, g2b_sb, "g2")

        # logits in [b, k] layout: lhsT = h2p [GHID, BS], rhs = g3w [GHID, K]
        zg3 = spsum.tile([BS, K], F32, tag="g")
        nc.tensor.matmul(zg3, lhsT=h2p, rhs=g3w_sb, start=True, stop=False)
        nc.tensor.matmul(zg3, lhsT=ones, rhs=g3b_sb, start=False, stop=True)

        # softmax along free dim (K)
        negmx = gat.tile([BS, 1], F32)
        nc.vector.tensor_reduce(negmx, zg3, mybir.AxisListType.X, OP.max, negate=True)
        e3t = gat.tile([BS, K], F32)
        ssum = gat.tile([BS, 1], F32)
        nc.scalar.activation(
            e3t, zg3, AF.Exp, bias=negmx[:, 0:1], accum_out=ssum[:, 0:1]
        )
        rcp = gat.tile([BS, 1], F32)
        nc.vector.reciprocal(rcp, ssum)
        ewT = gat.tile([BS, K], F32)  # [b, k]
        nc.vector.tensor_scalar_mul(ewT, e3t, rcp[:, 0:1])

        # per-expert row at partition 0: ew_rows[0, k, :] = ewT[:, k].T
        ew_rows = gat.tile([1, K, BS], F32)
        for k in range(K):
            rp = spsum.tile([1, BS], F32, tag="g")
            nc.tensor.transpose(rp, ewT[:, k : k + 1], ident)
            nc.vector.tensor_copy(out=ew_rows[:, k, :], in_=rp)

        # broadcast rows, minus 1/8: cb[:, k, :] = (ew_k - 0.125) bf16,
        # replicated over all 128 partitions
        cb = gat.tile([128, K, BS], BF16)
        xk1 = xkp.tile([120, K, 4, BS], BF16, tag="xk", name="xk0")
        for k in range(K):
            bp = spsum.tile([128, BS], F32, tag="g")
            nc.tensor.matmul(
                bp, lhsT=ones, rhs=ew_rows[:, k, :], start=True, stop=True
            )
            nc.vector.tensor_scalar(cb[:, k, :], bp, -0.125, None, OP.add)
            nc.vector.tensor_tensor(
                xk1[:, k],
                xb1,
                cb[:120, k, None, :].to_broadcast((120, 4, BS)),
                OP.mult,
            )

        if any(with_bias):
            ewps_p = spsum.tile([K, BS], F32, tag="g")
            nc.tensor.transpose(ewps_p, ewT, ident)
            ew_sb = gat.tile([K, BS], F32)
            nc.vector.tensor_copy(out=ew_sb, in_=ewps_p)

        # ---------------- trunk ----------------
        def emit_xk(xk, xb, P, IT, it0, it1):
            # xk[:, k, it0:it1, :] = xb[:, it0:it1, :] * c_k   (bf16)
            n = it1 - it0
            for k in range(K):
                nc.vector.tensor_tensor(
                    xk[:, k, it0:it1],
                    xb[:, it0:it1],
                    cb[:P, k, None, :].to_broadcast((P, n, BS)),
                    OP.mult,
                )

        xb = xb1
        xk_cur = xk1
        for li, (P, IT, O, chunks) in enumerate((L1, L2, L3)):
            last = li == 2
            inv = float(inv_sw[li])
            xk = xk_cur
            if not last:
                nx_sb = xbp.tile([128, O // 128, BS], BF16, tag=f"xb{li + 2}")
                nxk = xkp.tile(
                    [128, K, O // 128, BS], BF16, tag="xk", name=f"xk{li + 1}"
                )
                nxdone = 0
            if b_ext[li] is not None:
                bl_sb = gat.tile([K, O], F32, tag=f"bias{li}")
                nc.sync.dma_start(bl_sb, b_ext[li][:])

            oc0 = 0
            y_out = []
            for ci, ocsz in enumerate(chunks):
                oc = slice(oc0, oc0 + ocsz)
                zp = zpsum.tile([BS, 512], F32, tag="z", name=f"zp{li}_{ci}")[:, :ocsz]
                started = False
                if b_ext[li] is not None:
                    nc.tensor.matmul(
                        zp, lhsT=ew_sb, rhs=bl_sb[:, oc], start=True, stop=False
                    )
                    started = True
                if li == 0:
                    # mean first: s1 is resident before gating completes
                    for it in range(IT):
                        nc.tensor.matmul(
                            zp, lhsT=xb[:, it, :], rhs=s_sb[li][:, it, oc],
                            start=not started and it == 0, stop=False,
                        )
                    started = True
                # deviation matmuls, weights streamed per (expert, chunk).
                # For L2/L3 the input it-tiles 0..IT/2-1 come from the previous
                # layer's first chunk, so run those for all experts first: they
                # are ready before the previous layer's second chunk finishes.
                if li > 0:
                    nc.sync.dma_start(s_sb[li][:, :, oc], s_ext[li][:, :, oc])
                w_sbs = []
                for k in range(K):
                    wtag = f"w{li}_{ci}" if li == 2 else f"w{li}"
                    w_sb = wp[li].tile(
                        [P, IT, ocsz], E3, tag=wtag, name=f"w{li}_{k}_{ci}"
                    )
                    nc.sync.dma_start(w_sb, w_ext[li][k][:, :, oc])
                    w_sbs.append(w_sb)
                    if li == 0:
                        for it in range(IT):
                            nc.tensor.matmul(
                                zp, lhsT=xk[:, k, it, :], rhs=w_sb[:, it, :],
                                start=not started and k == 0 and it == 0,
                                stop=k == K - 1 and it == IT - 1,
                            )
                        started = True
                if li > 0:
                    # dev pass A (early it-tiles)
                    for k in range(K):
                        for it in range(IT // 2):
                            nc.tensor.matmul(
                                zp, lhsT=xk[:, k, it, :], rhs=w_sbs[k][:, it, :],
                                start=not started and k == 0 and it == 0,
                                stop=False,
                            )
                        started = True
                    # mean in the middle (weights landed long ago)
                    for it in range(IT):
                        nc.tensor.matmul(
                            zp, lhsT=xb[:, it, :], rhs=s_sb[li][:, it, oc],
                            start=False, stop=False,
                        )
                    # dev pass B closes the group
                    for k in range(K):
                        for it in range(IT // 2, IT):
                            nc.tensor.matmul(
                                zp, lhsT=xk[:, k, it, :], rhs=w_sbs[k][:, it, :],
                                start=False,
                                stop=it == IT - 1 and k == K - 1,
                            )

                if last:
                    y_sb = hpool.tile([BS, 512], F32, tag="y", name="y_sb")[:, :ocsz]
                    nc.vector.tensor_copy(out=y_sb, in_=zp)
                    y_out.append((y_sb, oc))
                else:
                    # h = (relu(z/sw) + exp(min(z/sw, 0))) - 1   (= elu(z/sw))
                    r = hscr.tile([BS, 512], F32, tag="hr", name="hr")[:, :ocsz]
                    nc.scalar.activation(r, zp, AF.Relu, scale=inv)
                    m = hscr.tile([BS, 512], F32, tag="hm", name="hm")[:, :ocsz]
                    nc.vector.tensor_scalar(m, zp, inv, 0.0, OP.mult, OP.min)
                    e = hscr.tile([BS, 512], F32, tag="he", name="he")[:, :ocsz]
                    nc.scalar.activation(e, m, AF.Exp)
                    hp1 = hscr.tile([BS, 512], F32, tag="hp", name="hp")[:, :ocsz]
                    nc.vector.tensor_tensor(hp1, r, e, OP.add)
                    h = hpool.tile([BS, 512], BF16, tag="hh", name="hh")[:, :ocsz]
                    if li == 0:
                        nc.vector.tensor_scalar(h, hp1, -1.0, None, OP.add)
                    else:
                        # also pre-scale layer-3 inputs by 1/s_w3 so the final
                        # PSUM holds unscaled y and can DMA straight out
                        nc.vector.tensor_scalar(
                            h, hp1, -1.0, float(inv_sw[2]), OP.add, OP.mult
                        )
                    # transpose each 128-col block into next layer's input layout
                    for j in range(ocsz // 128):
                        tp = tpsum.tile([128, BS], BF16, tag="tr")
                        nc.tensor.transpose(tp, h[:, j * 128 : (j + 1) * 128], identb)
                        nc.vector.tensor_copy(
                            out=nx_sb[:, (oc0 // 128) + j, :], in_=tp
                        )
                    # next layer's scale-before for the it-tiles just produced
                    emit_xk(nxk, nx_sb, 128, O // 128, nxdone, nxdone + ocsz // 128)
                    nxdone += ocsz // 128
                oc0 += ocsz
            for y_sb, oc in y_out:
                nc.sync.dma_start(y_ext[:, oc], y_sb)
            if not last:
                xb = nx_sb
                xk_cur = nxk

    nc.compile()
    return nc


def TileCtx(nc):
    return tile.TileContext(nc)


_PROG_CACHE: dict = {}


def _get_program(with_bias, inv_sw):
    key = (tuple(with_bias), tuple(inv_sw))
    if key not in _PROG_CACHE:
        _PROG_CACHE[key] = _build_program(tuple(with_bias), tuple(inv_sw))
    return _PROG_CACHE[key]


def _layout_w(W, P, IT):
    # [O, I] -> [P, IT, O] with element [p,it,o] = W[o,it*P+p]
    O, I = W.shape
    return W.T.reshape(IT, P, O).transpose(1, 0, 2)


def _prep_layer(W, P, IT):
    """Returns (dev_e3m4 [K,P,IT,O], mean_bf16 [P,IT,O], s_w)."""
    Kk, O, I = W.shape
    sw = float(2.0 ** np.floor(np.log2(15.0 / np.abs(W).max())))
    dev = np.stack([_layout_w(W[k] * sw, P, IT) for k in range(Kk)])
    dev = np.ascontiguousarray(dev.astype(ml_dtypes.float8_e3m4))
    mean = np.ascontiguousarray(
        _layout_w(W.mean(0) * sw, P, IT).astype(ml_dtypes.bfloat16)
    )
    return dev, mean, sw


def kernel(
    x_main, x_gate, g1_w, g1_b, g2_w, g2_b, g3_w, g3_b,
    W1, b1, W2, b2, W3, b3,
):
    x_main = np.asarray(x_main, np.float32)
    x_gate = np.asarray(x_gate, np.float32)
    g1_w = np.asarray(g1_w, np.float32)
    g1_b = np.asarray(g1_b, np.float32)
    g2_w = np.asarray(g2_w, np.float32)
    g2_b = np.asarray(g2_b, np.float32)
    g3_w = np.asarray(g3_w, np.float32)
    g3_b = np.asarray(g3_b, np.float32)
    W1 = np.asarray(W1, np.float32)
    b1 = np.asarray(b1, np.float32)
    W2 = np.asarray(W2, np.float32)
    b2 = np.asarray(b2, np.float32)
    W3 = np.asarray(W3, np.float32)
    b3 = np.asarray(b3, np.float32)

    with_bias = (bool(b1.any()), bool(b2.any()), bool(b3.any()))

    w1d, s1m, sw1 = _prep_layer(W1, 120, 4)
    w2d, s2m, sw2 = _prep_layer(W2, 128, 8)
    w3d, s3m, sw3 = _prep_layer(W3, 128, 8)
    inv_sw = (1.0 / sw1, 1.0 / sw2, 1.0 / sw3)

    nc = _get_program(with_bias, inv_sw)

    # gating blob (shared columns; xg filled per core)
    gblob = np.zeros((128, GBLOB_COLS), np.float32)
    gblob[:, BS : BS + 64] = g1_w.T
    gblob[0:64, BS + 64 : BS + 128] = g2_w.T
    gblob[0:64, BS + 128 : BS + 136] = g3_w.T
    gblob[0:64, BS + 136] = g1_b
    gblob[0:64, BS + 137] = g2_b - g2_w.sum(1)
    gblob[0, BS + 138 : BS + 146] = g3_b - g3_w.sum(1)

    shared = {
        "w1": w1d, "s1": s1m,
        "w2": w2d, "s2": s2m,
        "w3": w3d, "s3": s3m,
    }
    for name, b, flag, sw in (
        ("b1", b1, with_bias[0], sw1),
        ("b2", b2, with_bias[1], sw2),
        ("b3", b3, with_bias[2], sw3),
    ):
        if flag:
            shared[name] = np.ascontiguousarray(b * sw)

    in_maps = []
    for s in range(NCORES):
        xm_s = x_main[s * BS : (s + 1) * BS].T  # [480, BS]
        xm_s = np.ascontiguousarray(
            xm_s.reshape(4, 120, BS).transpose(1, 0, 2)
        )  # [120, 4, BS]
        gb_s = gblob.copy()
        gb_s[:, 0:BS] = x_gate[s * BS : (s + 1) * BS].T
        in_maps.append({**shared, "xm": xm_s, "gb": np.ascontiguousarray(gb_s)})

    global _last_in_maps
    _last_in_maps = in_maps
    res = run_bass_kernel_spmd(nc, in_maps, list(range(NCORES))).results
    return np.concatenate([res[s]["y"] for s in range(NCORES)], axis=0)


_last_in_maps = None
